# revision 1
# baseline (speedup 1.0000x reference)
"""GCN (3-layer GCNConv + 3 FC + log_softmax) on 8 Trainium2 NeuronCores.

Strategy
--------
Nodes are sharded across the 8 cores (6272 rows each, node count padded
50000 -> 50176).  Per conv layer:

  z = y_prev @ W          (computed on each core for its own rows, bf16)
  AllGather z             (every core gets the full 50176x128 bf16 z-table)
  agg[dst] = sum coef * z[src]   (per-core over the edges whose dst it owns)
  y = relu(agg + b [+ y1])

The sparse aggregation is done as:
  - edges sorted by destination tile (128 dst nodes per tile), padded per
    (tile, src-range-side) to multiples of 128
  - `dma_gather` fetches the 128 source rows of each chunk from the z-table
    in HBM into SBUF ([128 edges x 128 feat], bf16).  int16 gather indices
    only address 32768 rows, so edges are split into src<32768 / src>=32768
    sides gathered from two base offsets.
  - a one-hot scatter matrix S [128 edges x 128 dst] with S[e, dst_e]=coef_e
    is built on the vector engine with one fused tensor_scalar op
    (iota == dst_local) * coef
  - TensorE computes psum[f, d] += G^T S, accumulating all chunks of a tile.
The structure (chunk counts per tile/side) is made identical across cores
(cross-core max, padded with coef=0 dummy edges) so a single SPMD program
serves all 8 cores; per-core data (indices, coefs) are kernel inputs.

The FC head runs per tile, fully on-chip, with log_softmax over the free dim.
"""
import math
from dataclasses import dataclass, field

import numpy as np
import ml_dtypes

BF16 = ml_dtypes.bfloat16


@dataclass
class GCfg:
    n: int = 50000              # real node count
    n_feat: int = 128
    n_cores: int = 8
    tiles_per_core: int = 49
    group: int = 4              # dst tiles per psum group (<=4, 4*128 fp32 = 1 bank)
    split: int = 32768          # int16 gather address split
    n_classes: int = 40
    n_layers: int = 3

    @property
    def nsh(self):
        return self.tiles_per_core * 128

    @property
    def npad(self):
        return self.nsh * self.n_cores

    @property
    def n_groups(self):
        return math.ceil(self.tiles_per_core / self.group)


@dataclass
class Plan:
    cfg: GCfg
    # uniform structure (identical across cores)
    k: np.ndarray              # [tiles, 2] chunks per (tile, side)
    groups: list = field(default_factory=list)
    idx_cols: int = 0
    n_chunks: int = 0
    # per-core data
    eidx: list = field(default_factory=list)    # [128, idx_cols] int16
    edst: list = field(default_factory=list)    # [128, n_chunks] bf16
    ecoef: list = field(default_factory=list)   # [128, n_chunks] bf16


def preprocess(edge_index: np.ndarray, cfg: GCfg) -> Plan:
    n, nsh, npad = cfg.n, cfg.nsh, cfg.npad
    T, G = cfg.tiles_per_core, cfg.group
    NC = cfg.n_cores

    loop = np.arange(n, dtype=np.int64)
    src = np.concatenate([edge_index[0].astype(np.int64), loop])
    dst = np.concatenate([edge_index[1].astype(np.int64), loop])
    deg = np.bincount(dst, minlength=npad).astype(np.float32)
    deg[deg == 0] = 1.0
    norm = 1.0 / np.sqrt(deg)
    coef = (norm[src] * norm[dst]).astype(np.float32)

    core = dst // nsh
    tile = (dst % nsh) // 128
    dloc = dst % 128
    HA = (T + 1) // 2          # tiles in half a
    RA = HA * 128              # rows per core in half a
    RB = nsh - RA
    side = ((src % nsh) >= RA).astype(np.int64)
    grp = tile // G

    # sort edges by (core, group, side, tile)
    order = np.lexsort((tile, side, grp, core))
    src, dst, coef = src[order], dst[order], coef[order]
    core, tile, dloc, side = core[order], tile[order], dloc[order], side[order]
    grp = tile // G

    # counts per (core, tile, side)
    bid = (core * T + tile) * 2 + side
    cnt = np.bincount(bid, minlength=NC * T * 2).reshape(NC, T, 2)
    k = np.ceil(cnt.max(axis=0) / 128).astype(np.int64)   # [T, 2]
    zero = (k.sum(axis=1) == 0)
    k[zero, 0] = 1

    # build group structure (identical for all cores)
    groups = []
    idx_col = 0
    chunk_id = 0
    for g in range(cfg.n_groups):
        tiles = list(range(g * G, min((g + 1) * G, T)))
        ginfo = {"tiles": []}
        # gather-order: side 0 tiles, then side 1 tiles
        run_off = {0: None, 1: None}
        run_rows = {0: 0, 1: 0}
        meta_cols = {}  # (t, s) -> (first chunk id, run slot offset)
        for s in (0, 1):
            run_off[s] = idx_col
            slot = 0
            for t in tiles:
                meta_cols[(t, s)] = (chunk_id, slot)
                chunk_id += int(k[t, s])
                slot += int(k[t, s])
            run_rows[s] = slot * 128
            idx_col += slot * 8      # 128 idx/chunk -> 8 int16 cols/chunk
        ginfo["lo_rows"], ginfo["hi_rows"] = run_rows[0], run_rows[1]
        ginfo["lo_off"], ginfo["hi_off"] = run_off[0], run_off[1]
        for q, t in enumerate(tiles):
            chunks = []
            for s in (0, 1):
                c0, sl0 = meta_cols[(t, s)]
                for j in range(int(k[t, s])):
                    chunks.append((s, sl0 + j, c0 + j))
            ginfo["tiles"].append({"t": t, "q": q, "chunks": chunks})
        groups.append(ginfo)
    n_chunks = chunk_id
    idx_cols = idx_col

    plan = Plan(cfg=cfg, k=k, groups=groups, idx_cols=idx_cols, n_chunks=n_chunks)

    # per-core packed arrays
    # bucket order inside a core: (group, side, tile) -> matches gather order
    okey = (grp * 2 + side) * T + tile            # note: grp*2+side then tile keeps (g,s,t) order
    starts = np.searchsorted(core * (cfg.n_groups * 2 * T) + okey,
                             np.arange(NC * cfg.n_groups * 2 * T))
    # edge ranks within their (core,tile,side) bucket
    sort_key = core * (cfg.n_groups * 2 * T) + okey
    bucket_start_per_edge = starts[sort_key]
    rank = np.arange(len(src)) - bucket_start_per_edge

    # chunk slot layout per (t, s): position offset of bucket within core's padded stream
    pos_off = np.zeros((T, 2), dtype=np.int64)
    off = 0
    for g in groups:
        for s_ in (0, 1):
            for tinfo in g["tiles"]:
                t = tinfo["t"]
                pos_off[t, s_] = off
                off += int(k[t, s_]) * 128
    total_pos = off  # == n_chunks * 128

    for c in range(NC):
        m = core == c
        pos = pos_off[tile[m], side[m]] + rank[m]
        idx_full = np.zeros(total_pos, dtype=np.int16)
        dst_full = np.zeros(total_pos, dtype=np.float32)
        coef_full = np.zeros(total_pos, dtype=np.float32)
        sm, sdm = src[m], side[m]
        sv = np.where(sdm == 0,
                      (sm // nsh) * RA + (sm % nsh),
                      (sm // nsh) * RB + (sm % nsh) - RA)
        idx_full[pos] = sv.astype(np.int16)
        dst_full[pos] = dloc[m]
        coef_full[pos] = coef[m]
        # idx wrap: position i -> partition i%16, col i//16, replicated x8
        a16 = idx_full.reshape(-1, 16).T            # [16, total/16]
        eidx = np.tile(a16, (8, 1)).astype(np.int16)
        # meta: chunk column, partition = position % 128
        edst = dst_full.reshape(-1, 128).T.astype(np.float32)   # [128, n_chunks]
        ecoef = coef_full.reshape(-1, 128).T.astype(np.float32)
        plan.eidx.append(np.ascontiguousarray(eidx))
        plan.edst.append(np.ascontiguousarray(edst))
        plan.ecoef.append(np.ascontiguousarray(ecoef))
    return plan


def build_kernel(plan: Plan, stop_after: str = "full", mock_ag: bool = False, gather_mode: str = "normal", n_queues: int = 4, wide: bool = False):
    import concourse.mybir as mybir
    import concourse.tile as tile
    from concourse import bacc
    from concourse.bass import ts

    cfg = plan.cfg
    NC, T, H = cfg.n_cores, cfg.tiles_per_core, cfg.n_feat
    NCL = cfg.n_classes
    NSH, NPAD, SPLIT = cfg.nsh, cfg.npad, cfg.split
    f32, bf16, i16 = mybir.dt.float32, mybir.dt.bfloat16, mybir.dt.int16
    EQ, MUL, ADD, SUB = (mybir.AluOpType.is_equal, mybir.AluOpType.mult,
                         mybir.AluOpType.add, mybir.AluOpType.subtract)
    AF = mybir.ActivationFunctionType

    nc = bacc.Bacc("TRN2", target_bir_lowering=False, debug=False,
                   num_devices=NC, num_swdge_queues=n_queues)

    din = {}
    def dram_in(name, shape, dt):
        din[name] = nc.dram_tensor(name, shape, dt, kind="ExternalInput")
        return din[name]

    eidx_d = dram_in("eidx", [128, plan.idx_cols], i16)
    edst_d = dram_in("edst", [128, plan.n_chunks], f32)
    ecoef_d = dram_in("ecoef", [128, plan.n_chunks], f32)
    xT_d = dram_in("xT", [128, NSH], bf16)
    w_d = [dram_in(f"w{i}", [H, H], bf16) for i in range(3)]
    b_d = [dram_in(f"b{i}", [H, 1], f32) for i in range(3)]
    fw1_d = dram_in("fw1", [H, H], bf16)
    fw2_d = dram_in("fw2", [H, H], bf16)
    fw3_d = dram_in("fw3", [H, NCL], bf16)
    fb1_d = dram_in("fb1", [H, 1], f32)
    fb2_d = dram_in("fb2", [H, 1], f32)
    fb3_d = dram_in("fb3", [128, NCL], f32)
    out_d = nc.dram_tensor("out", [NSH, NCL], f32, kind="ExternalOutput")

    HA = (T + 1) // 2
    RA, RB = HA * 128, NSH - HA * 128
    ag_in_a = [nc.dram_tensor(f"ag_ina{i}", [RA, H], bf16, kind="Internal")
               for i in range(3)]
    ag_in_b = [nc.dram_tensor(f"ag_inb{i}", [RB, H], bf16, kind="Internal")
               for i in range(3)]
    ag_out_a = [nc.dram_tensor(f"ag_outa{i}", [RA * NC, H], bf16, kind="Internal",
                               addr_space="Shared") for i in range(3)]
    ag_out_b = [nc.dram_tensor(f"ag_outb{i}", [RB * NC, H], bf16, kind="Internal",
                               addr_space="Shared") for i in range(3)]

    def z_dst(L, t):
        if t < HA:
            return ag_in_a[L].ap()[t * 128:(t + 1) * 128, :]
        return ag_in_b[L].ap()[(t - HA) * 128:(t - HA + 1) * 128, :]

    def emit_ag(L, half):
        i, o = (ag_in_a, ag_out_a) if half == 0 else (ag_in_b, ag_out_b)
        if mock_ag:
            nc.sync.dma_start(out=o[L].ap()[0:i[L].shape[0], :], in_=i[L].ap())
        else:
            nc.gpsimd.collective_compute(
                "AllGather", mybir.AluOpType.bypass,
                replica_groups=[list(range(NC))],
                ins=[i[L].ap()], outs=[o[L].ap()])

    max_lo = max(g["lo_rows"] for g in plan.groups) // 128
    max_hi = max(max(g["hi_rows"] for g in plan.groups) // 128, 1)

    with tile.TileContext(nc) as tc:
        with (
            tc.tile_pool(name="const", bufs=1) as cp,
            tc.tile_pool(name="glo", bufs=3 if wide else 2, space="SBUF") as glop,
            tc.tile_pool(name="ghi", bufs=3 if wide else 2) as ghip,
            tc.tile_pool(name="sb", bufs=4) as sp,
            tc.tile_pool(name="smat", bufs=8) as smp,
            tc.tile_pool(name="pagg", bufs=2, space="PSUM") as pagg,
            tc.tile_pool(name="pmm", bufs=2, space="PSUM") as pmm,
        ):
            # ---- resident constants ----
            eidx = cp.tile([128, plan.idx_cols], i16, tag="eidx")
            nc.sync.dma_start(out=eidx[:], in_=eidx_d.ap())
            edst = cp.tile([128, plan.n_chunks], f32, tag="edst")
            nc.sync.dma_start(out=edst[:], in_=edst_d.ap())
            ecoef = cp.tile([128, plan.n_chunks], f32, tag="ecoef")
            nc.sync.dma_start(out=ecoef[:], in_=ecoef_d.ap())
            xT = cp.tile([128, NSH], bf16, tag="xT")
            nc.sync.dma_start(out=xT[:], in_=xT_d.ap())
            ws = []
            for i in range(3):
                w = cp.tile([H, H], bf16, tag=f"w{i}")
                nc.sync.dma_start(out=w[:], in_=w_d[i].ap())
                ws.append(w)
            bs = []
            for i in range(3):
                b = cp.tile([H, 1], f32, tag=f"b{i}")
                nc.sync.dma_start(out=b[:], in_=b_d[i].ap())
                bs.append(b)
            fw1 = cp.tile([H, H], bf16, tag="fw1")
            nc.sync.dma_start(out=fw1[:], in_=fw1_d.ap())
            fw2 = cp.tile([H, H], bf16, tag="fw2")
            nc.sync.dma_start(out=fw2[:], in_=fw2_d.ap())
            fw3 = cp.tile([H, NCL], bf16, tag="fw3")
            nc.sync.dma_start(out=fw3[:], in_=fw3_d.ap())
            fb1 = cp.tile([H, 1], f32, tag="fb1")
            nc.sync.dma_start(out=fb1[:], in_=fb1_d.ap())
            fb2 = cp.tile([H, 1], f32, tag="fb2")
            nc.sync.dma_start(out=fb2[:], in_=fb2_d.ap())
            fb3 = cp.tile([128, NCL], f32, tag="fb3")
            nc.sync.dma_start(out=fb3[:], in_=fb3_d.ap())

            iota = cp.tile([128, 128], bf16, tag="iota")
            nc.gpsimd.iota(iota[:], pattern=[[1, 128]], base=0,
                           channel_multiplier=0,
                           allow_small_or_imprecise_dtypes=True)
            y1 = cp.tile([128, NSH], bf16, tag="y1")

            # ---- z0 = x @ W0 (own rows), scatter to ag_in[0] ----
            for t in range(T):
                psz = pmm.tile([128, H], f32, tag="pz")
                nc.tensor.matmul(out=psz[:], lhsT=xT[:, ts(t, 128)],
                                 rhs=ws[0][:], start=True, stop=True)
                zt = sp.tile([128, H], bf16, tag="zt")
                nc.scalar.activation(out=zt[:], in_=psz[:], func=AF.Copy)
                nc.sync.dma_start(out=z_dst(0, t), in_=zt[:])
                if t == HA - 1:
                    emit_ag(0, 0)
            emit_ag(0, 1)

            gq = [0]
            def next_q():
                q = gq[0] % n_queues
                gq[0] += 1
                return q

            # ---- conv layers ----
            n_layers = {"z0": 0, "L0agg": 1, "L0": 1, "L1": 2}.get(stop_after, 3)
            for L in range(n_layers):
                ztab_a = ag_out_a[L].ap()
                ztab_b = ag_out_b[L].ap()
                for g in plan.groups:
                    glo = glop.tile([128, max_lo, 128], bf16, tag="glo")
                    nlo = g["lo_rows"] // 128
                    if gather_mode == "nog":
                        nc.vector.tensor_copy(out=glo[:, 0, :], in_=iota[:])
                    else:
                     nc.gpsimd.dma_gather(
                        out_ap=glo[:, :nlo, :], in_ap=ztab_a[:, :],
                        idxs_ap=eidx[:, g["lo_off"]:g["lo_off"] + nlo * 8],
                        num_idxs=g["lo_rows"], num_idxs_reg=g["lo_rows"],
                        elem_size=H, single_packet=False, queue_num=next_q())
                    nhi = g["hi_rows"] // 128
                    if nhi:
                        ghi = ghip.tile([128, max_hi, 128], bf16, tag="ghi")
                        if gather_mode == "nog":
                            nc.vector.tensor_copy(out=ghi[:, 0, :], in_=iota[:])
                        else:
                         nc.gpsimd.dma_gather(
                            out_ap=ghi[:, :nhi, :], in_ap=ztab_b[:, :],
                            idxs_ap=eidx[:, g["hi_off"]:g["hi_off"] + nhi * 8],
                            num_idxs=g["hi_rows"], num_idxs_reg=g["hi_rows"],
                            elem_size=H, single_packet=False, queue_num=next_q())
                    ps = pagg.tile([128, 512], f32, tag="pagg")
                    if gather_mode == "gonly":
                        S0 = smp.tile([128, 128], bf16, tag="S")
                        nc.vector.tensor_scalar(
                            out=S0[:], in0=iota[:], scalar1=edst[:, 0:1],
                            scalar2=ecoef[:, 0:1], op0=EQ, op1=MUL)
                        nc.tensor.matmul(out=ps[:, 0:128], lhsT=glo[:, 0, :],
                                         rhs=S0[:], start=True, stop=True,
                                         skip_group_check=True)
                        if nhi:
                            nc.tensor.matmul(out=ps[:, 128:256], lhsT=ghi[:, 0, :],
                                             rhs=S0[:], start=True, stop=True,
                                             skip_group_check=True)
                        continue
                    for tinfo in g["tiles"]:
                        t, q = tinfo["t"], tinfo["q"]
                        chunks = tinfo["chunks"]
                        for j, (s, slot, mc) in enumerate(chunks):
                            S = smp.tile([128, 128], bf16, tag="S")
                            nc.vector.tensor_scalar(
                                out=S[:], in0=iota[:],
                                scalar1=edst[:, mc:mc + 1],
                                scalar2=ecoef[:, mc:mc + 1],
                                op0=EQ, op1=MUL)
                            gb = glo if s == 0 else ghi
                            nc.tensor.matmul(
                                out=ps[:, ts(q, 128)],
                                lhsT=gb[:, slot, :], rhs=S[:],
                                start=(j == 0), stop=(j == len(chunks) - 1),
                                skip_group_check=True)
                        # ---- epilogue for tile t ----
                        if L == 0:
                            nc.scalar.activation(
                                out=y1[:, ts(t, 128)], in_=ps[:, ts(q, 128)],
                                func=AF.Relu, bias=bs[0][:])
                            ysrc = y1[:, ts(t, 128)]
                        else:
                            tmp = sp.tile([128, 128], f32, tag="tmp")
                            nc.vector.tensor_tensor(
                                out=tmp[:], in0=ps[:, ts(q, 128)],
                                in1=y1[:, ts(t, 128)], op=ADD)
                            yt = sp.tile([128, 128], bf16, tag="yt")
                            nc.scalar.activation(out=yt[:], in_=tmp[:],
                                                 func=AF.Relu, bias=bs[L][:])
                            ysrc = yt[:]
                        if L < 2 and stop_after != "L0agg":
                            psz = pmm.tile([128, H], f32, tag="pz")
                            nc.tensor.matmul(out=psz[:], lhsT=ysrc,
                                             rhs=ws[L + 1][:],
                                             start=True, stop=True)
                            zt = sp.tile([128, H], bf16, tag="zt")
                            nc.scalar.activation(out=zt[:], in_=psz[:],
                                                 func=AF.Copy)
                            nc.sync.dma_start(out=z_dst(L + 1, t), in_=zt[:])
                            if t == HA - 1:
                                emit_ag(L + 1, 0)
                        elif stop_after == "full":
                            # ---- FC head, per tile ----
                            ph = pmm.tile([128, H], f32, tag="pz")
                            nc.tensor.matmul(out=ph[:], lhsT=fw1[:], rhs=ysrc,
                                             start=True, stop=True)
                            h1 = sp.tile([128, H], bf16, tag="h1")
                            nc.scalar.activation(out=h1[:], in_=ph[:],
                                                 func=AF.Relu, bias=fb1[:])
                            ph2 = pmm.tile([128, H], f32, tag="pz")
                            nc.tensor.matmul(out=ph2[:], lhsT=fw2[:], rhs=h1[:],
                                             start=True, stop=True)
                            h2 = sp.tile([128, H], bf16, tag="h2")
                            nc.scalar.activation(out=h2[:], in_=ph2[:],
                                                 func=AF.Relu, bias=fb2[:])
                            p3 = pmm.tile([128, NCL], f32, tag="pz")
                            nc.tensor.matmul(out=p3[:], lhsT=h2[:], rhs=fw3[:],
                                             start=True, stop=True)
                            sb3 = sp.tile([128, NCL], f32, tag="sb3")
                            nc.vector.tensor_tensor(out=sb3[:], in0=p3[:],
                                                    in1=fb3[:], op=ADD)
                            mneg = sp.tile([128, 1], f32, tag="mneg")
                            nc.vector.tensor_reduce(
                                out=mneg[:], in_=sb3[:],
                                axis=mybir.AxisListType.X,
                                op=mybir.AluOpType.max, negate=True)
                            ex = sp.tile([128, NCL], f32, tag="ex")
                            ssum = sp.tile([128, 1], f32, tag="ssum")
                            nc.scalar.activation(out=ex[:], in_=sb3[:],
                                                 func=AF.Exp, bias=mneg[:],
                                                 accum_out=ssum[:])
                            lg = sp.tile([128, 1], f32, tag="lg")
                            nc.scalar.activation(out=lg[:], in_=ssum[:],
                                                 func=AF.Ln)
                            ot = sp.tile([128, NCL], f32, tag="ot")
                            nc.vector.tensor_scalar(
                                out=ot[:], in0=sb3[:], scalar1=mneg[:],
                                scalar2=lg[:], op0=ADD, op1=SUB)
                            nc.sync.dma_start(out=out_d.ap()[ts(t, 128), :],
                                              in_=ot[:])
                if L < 2 and stop_after != "L0agg":
                    emit_ag(L + 1, 1)

    nc.compile()
    return nc


def make_in_maps(inputs, plan: Plan):
    cfg = plan.cfg
    NC, NSH, H = cfg.n_cores, cfg.nsh, cfg.n_feat
    x = np.asarray(inputs["x"], dtype=np.float32)
    xp = np.zeros((cfg.npad, H), dtype=np.float32)
    xp[:x.shape[0]] = x
    Wc = np.asarray(inputs["Wconv"], dtype=np.float32)
    bc = np.asarray(inputs["bconv"], dtype=np.float32)
    in_maps = []
    for c in range(NC):
        m = {
            "eidx": plan.eidx[c].view(np.int16) if plan.eidx[c].dtype == np.int16 else plan.eidx[c],
            "edst": plan.edst[c],
            "ecoef": plan.ecoef[c],
            "xT": np.ascontiguousarray(xp[c * NSH:(c + 1) * NSH].T).astype(BF16),
            "fw1": np.asarray(inputs["fc1_w"], np.float32).astype(BF16),
            "fw2": np.asarray(inputs["fc2_w"], np.float32).astype(BF16),
            "fw3": np.asarray(inputs["fc3_w"], np.float32).astype(BF16),
            "fb1": np.asarray(inputs["fc1_b"], np.float32).reshape(H, 1),
            "fb2": np.asarray(inputs["fc2_b"], np.float32).reshape(H, 1),
            "fb3": np.tile(np.asarray(inputs["fc3_b"], np.float32)[None, :], (128, 1)),
        }
        for i in range(3):
            m[f"w{i}"] = Wc[i].astype(BF16)
            m[f"b{i}"] = bc[i].reshape(H, 1).astype(np.float32)
        in_maps.append(m)
    return in_maps


_CACHE = {}


def kernel(**inputs) -> np.ndarray:
    cfg = GCfg()
    edge_index = np.asarray(inputs["edge_index"])
    plan = preprocess(edge_index, cfg)
    key = "full"
    if key not in _CACHE:
        _CACHE[key] = build_kernel(plan)
    nc = _CACHE[key]
    in_maps = make_in_maps(inputs, plan)
    from concourse.bass_utils import run_bass_kernel_spmd
    res = run_bass_kernel_spmd(nc, in_maps, core_ids=list(range(cfg.n_cores)))
    out = np.concatenate([res.results[c]["out"] for c in range(cfg.n_cores)], axis=0)
    return np.ascontiguousarray(out[:cfg.n, :cfg.n_classes].astype(np.float32))

